# revision 55
# baseline (speedup 1.0000x reference)
"""Multi-head attention + residual + LayerNorm kernel for 8 TRN2 NeuronCores.

Reference computation (B=2, S=2048, DIM=1024, H=16, DH=64):
    q = x @ Wq.T + bq ; k = x @ Wk.T + bk ; v = x @ Wv.T + bv     (per batch)
    attn_h = softmax((q_h @ k_h.T) / sqrt(DH))
    z_init[b,h,s,d] = attn_h @ v_h
    z = z_init.reshape(B, S, H*DH)      # "faithful" reshape WITHOUT the
                                        # [B,H,S,DH]->[B,S,H,DH] transpose!
    out = LayerNorm(x + z) * gamma + beta

Sharding: core c owns batch c//4 and heads 4r..4r+3 (r = c%4), i.e. output
rows 512r..512r+512.  No collectives; host does transposes/slices/concat.

Schedule: the attention inner loop is ACT-bound (exp of [128,1024] every
~1.15us vs ~0.65us of PE work), so everything else is *woven into* that
loop to keep both engines saturated -- and, critically, to keep the PE
dense enough that the HAM clock gate stays at 2.4 GHz (sparse PE phases
measurably re-throttle to 1.2 GHz and double every matmul):
  - xT / weights / xrow are bf16 (halves load BW + SBUF), DMA'd in
    need-by order (k weights, q weights, xT seq-chunks 0-1, v weights,
    xT 2-3, residual rows last).
  - projection chains pair two moving seq-chunks per stationary weight
    chunk, so only every other matmul pays an exposed LDWEIGHTS.
  - pair 1's projections are emitted interleaved (2 per tb block) inside
    pair 0's attention loop; each pair's second-half qT weaves into its
    own attention (only needed from sc2).
  - pair 0's scrambled-z transposes + LayerNorm weave inside pair 1's
    attention; z transposes batch 4 j-columns per psum tile so the
    normalize is one reciprocal + one broadcast multiply.
  - LayerNorm's 1/sqrt(var+eps) runs on the DVE (quake-style bit trick +
    3 Newton steps on [128,1] tiles): keeping Sqrt off the ACT engine pins
    its activation-table set to exp_and_others, so the 128 attention exps
    never pay a ~2.7us table reload.
Attention matmul operands stay f32r: a bf16 PV variant measured *slower*
(attn-phase PE went cold), despite identical instruction counts.
Per-core device dataflow is unchanged from v0: scores are computed
transposed sT[t,s]=k.q so softmax's reduction lands on the PV contraction
(ones column appended to v accumulates the denominator), zT blocks are
PE-transposed into the scrambled layout, LN runs rows-on-partitions.
"""

import itertools

import numpy as np
import ml_dtypes

import concourse.bass as bass
import concourse.bacc as bacc
import concourse.mybir as mybir
import concourse.tile as tile
from concourse import bass_utils
from concourse.masks import make_identity

F32 = mybir.dt.float32
F32R = mybir.dt.float32r
BF16 = mybir.dt.bfloat16

B, S, DIM, H = 2, 2048, 1024, 16
DH = DIM // H  # 64
N_CORES = 8
CPB = N_CORES // B   # cores per batch = 4
HPC = H // CPB       # heads per core = 4
SS = S // CPB        # output rows per core = 512
LN_EPS = 1e-5


def build_mha(nc: bass.Bass):
    s_keys, dim, hpc, dh = S, DIM, HPC, DH
    n_pairs = hpc // 2       # head pairs per core (2)
    kc = dim // 128          # contraction chunks (8)
    tb_n = s_keys // 128     # key blocks (16)
    sc_n = s_keys // 512     # 512-wide query chunks (4)
    mrows = s_keys // 16     # scrambled rows per head (128)
    ss_out = hpc * mrows     # output rows per core (512)
    wcols = hpc * dh         # per-core projection output cols (256)

    xT = nc.dram_tensor("xT", [dim, s_keys], BF16, kind="ExternalInput").ap()
    xrow = nc.dram_tensor("xrow", [ss_out, dim], BF16, kind="ExternalInput").ap()
    WqTs = nc.dram_tensor("WqTs", [dim, wcols], BF16, kind="ExternalInput").ap()
    WkTs = nc.dram_tensor("WkTs", [dim, wcols], BF16, kind="ExternalInput").ap()
    WvTs = nc.dram_tensor("WvTs", [dim, wcols], BF16, kind="ExternalInput").ap()
    bqs = nc.dram_tensor("bqs", [wcols], F32, kind="ExternalInput").ap()
    bks = nc.dram_tensor("bks", [wcols], F32, kind="ExternalInput").ap()
    bvs = nc.dram_tensor("bvs", [wcols], F32, kind="ExternalInput").ap()
    gamma = nc.dram_tensor("gamma", [dim], F32, kind="ExternalInput").ap()
    beta = nc.dram_tensor("beta", [dim], F32, kind="ExternalInput").ap()
    out = nc.dram_tensor("out", [ss_out, dim], F32, kind="ExternalOutput").ap()

    EXP = mybir.ActivationFunctionType.Exp

    with tile.TileContext(nc) as tc:
        with tc.tile_pool(name="singles", bufs=1) as singles, \
             tc.tile_pool(name="kpool", bufs=2) as kpool, \
             tc.tile_pool(name="qpool", bufs=2) as qpool, \
             tc.tile_pool(name="vstage", bufs=2) as vstage_pool, \
             tc.tile_pool(name="vpool", bufs=2) as vpool, \
             tc.tile_pool(name="epool", bufs=4) as epool, \
             tc.tile_pool(name="ztpool", bufs=4) as ztpool, \
             tc.tile_pool(name="rpool", bufs=4) as rpool, \
             tc.tile_pool(name="lnx", bufs=1) as lnx, \
             tc.tile_pool(name="lnw", bufs=2) as lnw, \
             tc.tile_pool(name="ps_pt", bufs=2, space="PSUM") as ps_pt, \
             tc.tile_pool(name="ps_sT", bufs=2, space="PSUM") as ps_sT, \
             tc.tile_pool(name="ps_z", bufs=2, space="PSUM") as ps_z:

            # ---- persistent tiles -------------------------------------------
            xT_sb = singles.tile([128, kc, s_keys], BF16)
            z_all = singles.tile([mrows, hpc, dim], F32)
            ident = singles.tile([128, 128], F32)
            make_identity(nc, ident)
            ones_col = singles.tile([128, 1], F32)
            nc.vector.memset(ones_col, 1.0)
            wq_all = singles.tile([128, kc, wcols], BF16)
            wk_all = singles.tile([128, kc, wcols], BF16)
            wv_all = singles.tile([128, kc, wcols], BF16)
            biases = singles.tile([128, 3, n_pairs], F32)
            grep = singles.tile([128, dim], F32)
            brep = singles.tile([128, dim], F32)

            # ---- DMAs in need-by order --------------------------------------
            # weights first (gate the first projection chains), full-width
            # rows (512B lines), then xT by seq-chunk (kT consumes it
            # t-chunk by t-chunk).
            def _w_dma(w_sb, wt, ch):
                cs = slice(ch * (kc // 2), (ch + 1) * (kc // 2))
                nc.sync.dma_start(
                    out=w_sb[:, cs, :],
                    in_=bass.AP(tensor=wt.tensor,
                                offset=ch * (kc // 2) * 128 * wcols,
                                ap=[[wcols, 128], [128 * wcols, kc // 2],
                                    [1, wcols]]))

            def _x_dma(t, ch):
                cs = slice(ch * (kc // 2), (ch + 1) * (kc // 2))
                nc.sync.dma_start(
                    out=xT_sb[:, cs, t * 512:(t + 1) * 512],
                    in_=bass.AP(
                        tensor=xT.tensor,
                        offset=ch * (kc // 2) * 128 * s_keys + t * 512,
                        ap=[[s_keys, 128], [128 * s_keys, kc // 2], [1, 512]]))

            for ch in range(2):
                _w_dma(wk_all, WkTs, ch)
            for ch in range(2):
                _w_dma(wq_all, WqTs, ch)
            for t in range(2):
                for ch in range(2):
                    _x_dma(t, ch)
            for ch in range(2):
                _w_dma(wv_all, WvTs, ch)
            for t in range(2, sc_n):
                for ch in range(2):
                    _x_dma(t, ch)
            for j, bt in enumerate((bqs, bks, bvs)):
                nc.gpsimd.dma_start(
                    out=biases[:, j, :],
                    in_=bass.AP(tensor=bt.tensor, offset=0,
                                ap=[[1, 128], [128, n_pairs]]))
            nc.gpsimd.dma_start(
                out=grep, in_=bass.AP(tensor=gamma.tensor, offset=0,
                                      ap=[[0, 128], [1, dim]]))
            nc.gpsimd.dma_start(
                out=brep, in_=bass.AP(tensor=beta.tensor, offset=0,
                                      ap=[[0, 128], [1, dim]]))
            # residual rows prefetch (read only by LN, which runs woven)
            xt4 = lnx.tile([mrows, hpc, dim], BF16, tag="xt4")
            for sb2 in range(2):
                nc.sync.dma_start(
                    out=xt4[:, 2 * sb2:2 * sb2 + 2, :],
                    in_=bass.AP(tensor=xrow.tensor,
                                offset=2 * sb2 * mrows * dim,
                                ap=[[dim, 128], [mrows * dim, 2], [1, dim]]))

            P = {}   # pl -> dict(qT=, kT=, v=)
            ZT = {}  # (pl, hl) -> zT accumulator tile

            def _alloc_pair(pl):
                if pl not in P:
                    P[pl] = dict(
                        kT=kpool.tile([128, s_keys], F32R, tag="kT",
                                      name=f"kT{pl}"),
                        qT=qpool.tile([128, s_keys], F32R, tag="qT",
                                      name=f"qT{pl}"),
                        v=vpool.tile([128, tb_n, 2, dh + 1], F32R,
                                     tag="v_sb", name=f"v{pl}"))
                return P[pl]

            def proj_chain(pl, which, t2):
                """One projection chain: 16 matmuls covering seq chunks
                (2*t2, 2*t2+1); consecutive matmuls share the stationary
                w[:, c] so only every other one pays LDWEIGHTS.  Yields once
                per PE op so the caller can weave."""
                wsl = slice(pl * 128, (pl + 1) * 128)
                pair = _alloc_pair(pl)
                dst, w_sb, bj = {
                    "k": (pair["kT"], wk_all, 1),
                    "q": (pair["qT"], wq_all, 0),
                    "v": (None, wv_all, 2),
                }[which]
                pss = [ps_pt.tile([128, 512], F32, tag="pt", name=f"ps{half}")
                       for half in range(2)]
                for c in range(kc):
                    for half in range(2):
                        t = 2 * t2 + half
                        nc.tensor.matmul(
                            pss[half], w_sb[:, c, wsl],
                            xT_sb[:, c, t * 512:(t + 1) * 512],
                            start=(c == 0), stop=(c == kc - 1))
                        yield
                for half in range(2):
                    t = 2 * t2 + half
                    if dst is not None:
                        nc.vector.tensor_scalar_add(
                            out=dst[:, t * 512:(t + 1) * 512], in0=pss[half],
                            scalar1=biases[:, bj, pl:pl + 1])
                    else:
                        # v path: bias into a staging tile, then 4
                        # transposes per psum tile -> [t, dv] blocks
                        vstg = vstage_pool.tile([128, 512], F32, tag="vstg")
                        nc.vector.tensor_scalar_add(
                            out=vstg, in0=pss[half],
                            scalar1=biases[:, 2, pl:pl + 1])
                        ptr = ps_pt.tile([128, 512], F32, tag="pt",
                                         name="ptr")
                        for j in range(4):
                            nc.tensor.transpose(
                                ptr[:, j * 128:(j + 1) * 128],
                                vstg[:, j * 128:(j + 1) * 128], ident)
                            yield
                        nc.vector.tensor_copy(
                            out=pair["v"][:, t * 4:(t + 1) * 4, :, 0:dh],
                            in_=ptr.rearrange("p (j h d) -> p j h d",
                                              j=4, h=2))

            def proj_main(pl):
                """Everything pair pl's attention needs from the start: full
                kT, first-half qT, full v (plus the denominator ones)."""
                for which, t2 in (("k", 0), ("k", 1), ("q", 0),
                                  ("v", 0), ("v", 1)):
                    yield from proj_chain(pl, which, t2)
                nc.vector.tensor_copy(
                    out=P[pl]["v"][:, :, :, dh:dh + 1],
                    in_=ones_col.to_broadcast([128, tb_n, 2, 1]))

            def proj_late(pl):
                """Second-half qT -- only needed from sc2, so it can weave
                into pair pl's own attention."""
                yield from proj_chain(pl, "q", 1)

            def ln_emit(sb):
                """Residual + LayerNorm for scrambled-row block sb (all DVE +
                two tiny ACT calls; safe to emit mid-attention)."""
                xz = lnw.tile([mrows, dim], F32, tag="xz")
                nc.vector.tensor_add(xz, xt4[:, sb, :], z_all[:, sb, :])
                st = lnw.tile([mrows, 2, 6], F32, tag="st")
                xz_g = xz.rearrange("p (g d) -> p g d", g=2)
                for g in range(2):
                    nc.vector.bn_stats(out=st[:, g, :], in_=xz_g[:, g, :])
                mv = lnw.tile([mrows, 2], F32, tag="mv")
                nc.vector.bn_aggr(out=mv, in_=st)
                # rstd = (var+eps)^-0.5 entirely on DVE (quake init + 3 Newton
                # steps on a [128,1] tile) -- keeps ACT's table set pinned to
                # exp_and_others so attention exps never reload tables.
                U32 = mybir.dt.uint32
                veps = lnw.tile([mrows, 1], F32, tag="veps")
                nc.vector.tensor_scalar_add(out=veps, in0=mv[:, 1:2],
                                            scalar1=LN_EPS)
                I32 = mybir.dt.int32
                y = lnw.tile([mrows, 1], F32, tag="rsq_y")
                # y0 bits = 0x5f3759df - (bits(veps) >> 1): shr (bitwise),
                # then negate+add (both arith; op0/op1 must share a class)
                nc.vector.tensor_scalar(
                    out=y.bitcast(I32), in0=veps.bitcast(I32),
                    scalar1=1, scalar2=None,
                    op0=mybir.AluOpType.logical_shift_right)
                nc.vector.tensor_scalar(
                    out=y.bitcast(I32), in0=y.bitcast(I32),
                    scalar1=-1, scalar2=0x5F3759DF,
                    op0=mybir.AluOpType.mult, op1=mybir.AluOpType.add)
                t = lnw.tile([mrows, 1], F32, tag="rsq_t")
                for _ in range(3):
                    nc.vector.tensor_mul(t, y, y)
                    nc.vector.tensor_mul(t, t, veps)
                    nc.vector.tensor_scalar(
                        out=t, in0=t, scalar1=-0.5, scalar2=1.5,
                        op0=mybir.AluOpType.mult, op1=mybir.AluOpType.add)
                    nc.vector.tensor_mul(y, y, t)
                xn = lnw.tile([mrows, dim], F32, tag="xn")
                nc.vector.tensor_scalar(
                    out=xn, in0=xz, scalar1=mv[:, 0:1], scalar2=y,
                    op0=mybir.AluOpType.subtract, op1=mybir.AluOpType.mult)
                nc.vector.tensor_mul(xn, xn, grep[:mrows])
                nc.vector.tensor_add(xn, xn, brep[:mrows])
                nc.sync.dma_start(out=out[sb * mrows:(sb + 1) * mrows, :],
                                  in_=xn)

            def ztrans_units(pl):
                """Scrambled-z transposes + LN for pair pl; yields per PE op.
                Four j-columns of [s,65] land in one psum tile so the
                normalize is one reciprocal + one broadcast-multiply.  Both
                LN chains are emitted only after ALL transposes: the DVE is
                in-order, so a ~7us LN chain emitted mid-stream would make
                later transpose-normalizes (and with them the shared psum
                transpose tiles the PE is waiting on) queue behind it."""
                for hl in range(2):
                    sb = 2 * pl + hl
                    zth = ZT[(pl, hl)]
                    zin_all = zth.rearrange("p (m j) -> p j m", j=16)
                    for j4 in range(4):
                        ptz = ps_pt.tile([128, 4, dh + 1], F32, tag="pt",
                                         name="ptz")
                        for jj in range(4):
                            nc.tensor.transpose(
                                ptz[:mrows, jj, :], zin_all[:, j4 * 4 + jj, :],
                                ident[0:dh + 1, 0:dh + 1])
                            yield
                        rc = rpool.tile([mrows, 4, 1], F32, tag="recip")
                        nc.vector.reciprocal(rc, ptz[:mrows, :, dh:dh + 1])
                        nc.vector.tensor_tensor(
                            out=z_all.rearrange(
                                "p h (j d) -> p h j d", j=16)
                            [:, sb, j4 * 4:(j4 + 1) * 4, :],
                            in0=ptz[:mrows, :, 0:dh],
                            in1=rc.to_broadcast([mrows, 4, dh]),
                            op=mybir.AluOpType.mult)
                for hl in range(2):
                    ln_emit(2 * pl + hl)
                    yield

            def attention(pl, bg, weave_fn):
                """Score->exp->PV loop for pair pl, weaving background PE ops
                from generator bg between tb blocks.  Software-pipelined by
                one step: scores+exp for step g+1 are emitted BEFORE PV of
                step g, so the exp (the pacing engine) is never queued behind
                PV matmuls or woven background work."""
                qT, kT, v_sb = P[pl]["qT"], P[pl]["kT"], P[pl]["v"]
                ets = {}
                zps_by_sc = {}

                def scores_exp(g):
                    sc, tb = divmod(g, tb_n)
                    ssp = ps_sT.tile([128, 1024], F32, tag="sT")
                    for hl in range(2):
                        hsl = slice(64 * hl, 64 * hl + 64)
                        nc.tensor.matmul(
                            ssp[:, hl * 512:(hl + 1) * 512],
                            kT[hsl, tb * 128:(tb + 1) * 128],
                            qT[hsl, sc * 512:(sc + 1) * 512],
                            start=True, stop=True)
                    et = epool.tile([128, 1024], F32R, tag="expT")
                    nc.scalar.activation(out=et, in_=ssp, func=EXP,
                                         scale=0.125)
                    ets[g] = et

                scores_exp(0)
                for g in range(sc_n * tb_n):
                    sc, tb = divmod(g, tb_n)
                    if g + 1 < sc_n * tb_n:
                        scores_exp(g + 1)
                    if tb == 0:
                        zps_by_sc[sc] = [
                            ps_z.tile([dh + 1, 512], F32, tag="zacc",
                                      name=f"zacc_{pl}_{sc}_{hl}")
                            for hl in range(2)]
                    zps = zps_by_sc[sc]
                    et = ets.pop(g)
                    for hl in range(2):
                        nc.tensor.matmul(
                            zps[hl], v_sb[:, tb, hl, :],
                            et[:, hl * 512:(hl + 1) * 512],
                            start=(tb == 0), stop=(tb == tb_n - 1))
                    for _ in range(weave_fn(sc, tb)):
                        if next(bg, None) is None:
                            break
                    if tb == tb_n - 1:
                        for hl in range(2):
                            if (pl, hl) not in ZT:
                                ZT[(pl, hl)] = ztpool.tile(
                                    [dh + 1, s_keys], F32, tag="ztsb",
                                    name=f"zth_{pl}_{hl}")
                            nc.vector.tensor_copy(
                                out=ZT[(pl, hl)][:, sc * 512:(sc + 1) * 512],
                                in_=zps[hl])

            def _drain(gen):
                for _ in gen:
                    pass

            # pair 0's startup projections (nothing to hide them behind yet);
            # its second-half qT defers into its own attention loop.
            _drain(proj_main(0))
            # pair 0 attention, weaving its late qT then pair 1's projections
            # (112 units at 2/tb, spread evenly to keep PE density up for the
            # HAM clock gate); hold off the first few tb so the
            # score->exp->PV pipeline fills first.
            bg1 = itertools.chain(proj_late(0), proj_main(1))
            attention(0, bg1,
                      lambda sc, tb: 0 if (sc == 0 and tb < 6) else 2)
            _drain(bg1)
            # pair 1 attention, weaving its late qT, then pair 0's
            # z-transposes + LN(0), LN(1)
            bg2 = itertools.chain(proj_late(1), ztrans_units(0))
            attention(1, bg2, lambda sc, tb: 1)
            _drain(bg2)
            # tail: pair 1's z-transposes + LN(2), LN(3)
            _drain(ztrans_units(1))

    return nc


def _shard_inputs(embedded, Wq, bq, Wk, bk, Wv, bv, gamma, beta):
    """Host-side sharding: transposes / slices / casts / concatenation only."""
    embedded = np.asarray(embedded, dtype=np.float32)
    c = np.ascontiguousarray
    bf = ml_dtypes.bfloat16
    WqT = np.asarray(Wq, dtype=np.float32).T
    WkT = np.asarray(Wk, dtype=np.float32).T
    WvT = np.asarray(Wv, dtype=np.float32).T
    bq = np.asarray(bq, np.float32)
    bk = np.asarray(bk, np.float32)
    bv = np.asarray(bv, np.float32)
    gb = {
        "gamma": c(np.asarray(gamma, np.float32)),
        "beta": c(np.asarray(beta, np.float32)),
    }
    xT_by_batch = [c(embedded[b].T.astype(bf)) for b in range(B)]
    in_maps = []
    for core in range(N_CORES):
        b, r = core // CPB, core % CPB
        rows = slice(r * SS, (r + 1) * SS)
        cols = slice(r * HPC * DH, (r + 1) * HPC * DH)
        in_maps.append({
            "xT": xT_by_batch[b],
            "xrow": c(embedded[b, rows].astype(bf)),
            "WqTs": c(WqT[:, cols].astype(bf)),
            "WkTs": c(WkT[:, cols].astype(bf)),
            "WvTs": c(WvT[:, cols].astype(bf)),
            "bqs": c(bq[cols]), "bks": c(bk[cols]), "bvs": c(bv[cols]),
            **gb,
        })
    return in_maps


_BUILT = {}


def _get_nc():
    if "nc" not in _BUILT:
        nc = bacc.Bacc("TRN2", debug=False, target_bir_lowering=False)
        build_mha(nc)
        nc.compile()
        _BUILT["nc"] = nc
    return _BUILT["nc"]


def kernel(embedded, Wq, bq, Wk, bk, Wv, bv, gamma, beta, _trace=False):
    nc = _get_nc()
    in_maps = _shard_inputs(embedded, Wq, bq, Wk, bk, Wv, bv, gamma, beta)
    res = bass_utils.run_bass_kernel_spmd(
        nc, in_maps, core_ids=list(range(N_CORES)), trace=_trace)
    outs = [r["out"] for r in res.results]
    full = np.stack([
        np.concatenate(outs[b * CPB:(b + 1) * CPB], axis=0) for b in range(B)
    ])
    if _trace:
        kernel._last_results = res
    return full.astype(np.float32)
